# revision 3
# baseline (speedup 1.0000x reference)
"""Trainium2 Bass kernel for nn_DMFLodel_8272107012191 (calibrated loss_fn).

Math (reference):
    occ      = max(class_occ, 1e-8)                      # [C]
    cal      = exp(logit - occ**-0.25)                   # [B, C]
    y_logit  = cal[b, y[b]]                              # [B]
    z_target = logit[b, y[b]]                            # [B]
    denom    = y_logit + sum_c(logit[b, :]) - z_target   # [B]
    loss     = mean_b(-log(divide_no_nan(y_logit, denom)))

Key observation: the full [B, C] calibrated matrix is never needed — only
the row sums of the raw logits plus two per-row gathers.  That makes this
a pure streaming-reduction problem: read logit once (206 MB total), plus
O(B) gathered elements.

Sharding: data-parallel over the batch axis.  8 cores x 128 rows each;
one batch row per SBUF partition.  Each core:
  1. streams its [128, C] logit shard through SBUF in large chunks,
     reducing each chunk over the free axis on the vector engine,
  2. gathers logit[b, y[b]] and class_occ[y[b]] via indirect DMA,
  3. computes the per-row loss in a handful of [128, 1] vector/scalar ops,
  4. writes the [128, 1] per-row loss back to DRAM.
The host concatenates the 8x128 per-row losses and takes the mean
(the all-reduce of the scalar loss collapses to this host-side mean).
"""

import numpy as np

B = 1024
C = 50257
N_CORES = 8
B_SH = B // N_CORES  # 128 rows per core == one SBUF partition each
TAU = 1.0
EPS = 1e-8

# streaming chunk width along the class axis (32 KiB/partition, 4 MiB/DMA)
CHUNK_W = 8192

_compiled = None  # (nc module, run fn) cache


def _build_nc():
    import concourse.bacc as bacc
    import concourse.bass as bass
    import concourse.tile as tile
    from concourse import mybir

    f32 = mybir.dt.float32
    i32 = mybir.dt.int32
    ACT = mybir.ActivationFunctionType

    nc = bacc.Bacc(
        "TRN2", target_bir_lowering=False, debug=False, num_devices=N_CORES
    )

    logit = nc.dram_tensor("logit", [B_SH, C], f32, kind="ExternalInput")
    yflat = nc.dram_tensor("yflat", [B_SH, 1], i32, kind="ExternalInput")
    y32 = nc.dram_tensor("y32", [B_SH, 1], i32, kind="ExternalInput")
    occ = nc.dram_tensor("class_occ", [C, 1], f32, kind="ExternalInput")
    loss_out = nc.dram_tensor("loss", [B_SH, 1], f32, kind="ExternalOutput")

    chunks = []
    off = 0
    while off < C:
        w = min(CHUNK_W, C - off)
        chunks.append((off, w))
        off += w
    n_chunks = len(chunks)

    with tile.TileContext(nc) as tc:
        with (
            tc.tile_pool(name="stream", bufs=3) as stream_pool,
            tc.tile_pool(name="small", bufs=1) as small,
        ):
            partials = small.tile([B_SH, n_chunks], f32)
            for i, (coff, w) in enumerate(chunks):
                t = stream_pool.tile([B_SH, CHUNK_W], f32, tag="stream")
                nc.sync.dma_start(out=t[:, :w], in_=logit[:, coff : coff + w])
                nc.vector.tensor_reduce(
                    out=partials[:, i : i + 1],
                    in_=t[:, :w],
                    axis=mybir.AxisListType.X,
                    op=mybir.AluOpType.add,
                )
            rowsum = small.tile([B_SH, 1], f32)
            nc.vector.tensor_reduce(
                out=rowsum[:],
                in_=partials[:],
                axis=mybir.AxisListType.X,
                op=mybir.AluOpType.add,
            )

            # per-row gathers: z_target = logit[b, y[b]], occ_y = class_occ[y[b]]
            yflat_t = small.tile([B_SH, 1], i32)
            nc.sync.dma_start(out=yflat_t[:], in_=yflat[:, :])
            y_t = small.tile([B_SH, 1], i32)
            nc.sync.dma_start(out=y_t[:], in_=y32[:, :])

            zt = small.tile([B_SH, 1], f32)
            nc.gpsimd.indirect_dma_start(
                out=zt[:],
                out_offset=None,
                in_=logit.ap().flatten().unsqueeze(1),
                in_offset=bass.IndirectOffsetOnAxis(ap=yflat_t[:, :1], axis=0),
            )
            occ_y = small.tile([B_SH, 1], f32)
            nc.gpsimd.indirect_dma_start(
                out=occ_y[:],
                out_offset=None,
                in_=occ.ap(),
                in_offset=bass.IndirectOffsetOnAxis(ap=y_t[:, :1], axis=0),
            )

            # pen = max(occ_y, EPS) ** -0.25  ==  1 / sqrt(sqrt(clamped))
            occ_c = small.tile([B_SH, 1], f32)
            nc.vector.tensor_scalar_max(out=occ_c[:], in0=occ_y[:], scalar1=EPS)
            s1 = small.tile([B_SH, 1], f32)
            nc.scalar.activation(out=s1[:], in_=occ_c[:], func=ACT.Sqrt)
            s2 = small.tile([B_SH, 1], f32)
            nc.scalar.activation(out=s2[:], in_=s1[:], func=ACT.Sqrt)
            neg_pen = small.tile([B_SH, 1], f32)
            nc.vector.reciprocal(out=neg_pen[:], in_=s2[:])
            nc.vector.tensor_scalar_mul(out=neg_pen[:], in0=neg_pen[:], scalar1=-TAU)

            # y_logit = exp(z_target - pen)
            ylog = small.tile([B_SH, 1], f32)
            nc.scalar.activation(out=ylog[:], in_=zt[:], func=ACT.Exp, bias=neg_pen[:])

            # denom = y_logit + (rowsum - z_target)
            den = small.tile([B_SH, 1], f32)
            nc.vector.tensor_tensor(
                out=den[:], in0=rowsum[:], in1=zt[:], op=mybir.AluOpType.subtract
            )
            nc.vector.tensor_tensor(
                out=den[:], in0=den[:], in1=ylog[:], op=mybir.AluOpType.add
            )

            # ratio = divide_no_nan(y_logit, denom)  (reciprocal + multiply)
            rden = small.tile([B_SH, 1], f32)
            nc.vector.reciprocal(out=rden[:], in_=den[:])
            ratio = small.tile([B_SH, 1], f32)
            nc.vector.tensor_tensor(
                out=ratio[:], in0=ylog[:], in1=rden[:], op=mybir.AluOpType.mult
            )
            mask = small.tile([B_SH, 1], mybir.dt.uint8)
            nc.vector.tensor_scalar(
                out=mask[:],
                in0=den[:],
                scalar1=0.0,
                scalar2=None,
                op0=mybir.AluOpType.is_equal,
            )
            zero = small.tile([B_SH, 1], f32)
            nc.vector.memset(zero[:], 0.0)
            nc.vector.copy_predicated(out=ratio[:], mask=mask[:], data=zero[:])

            # loss = -log(ratio)
            lnr = small.tile([B_SH, 1], f32)
            nc.scalar.activation(out=lnr[:], in_=ratio[:], func=ACT.Ln)
            nc.vector.tensor_scalar_mul(out=lnr[:], in0=lnr[:], scalar1=-1.0)
            nc.sync.dma_start(out=loss_out[:, :], in_=lnr[:])

    nc.compile()
    return nc


def _get_nc():
    global _compiled
    if _compiled is None:
        _compiled = _build_nc()
    return _compiled


def make_in_maps(logit, y, class_occ):
    """Build the 8 per-core input dicts from the full-size inputs."""
    logit = np.ascontiguousarray(np.asarray(logit, dtype=np.float32))
    y_i = np.asarray(y).astype(np.int32).reshape(B)
    occ_col = np.ascontiguousarray(
        np.asarray(class_occ, dtype=np.float32).reshape(C, 1)
    )
    row_base = np.arange(B_SH, dtype=np.int32) * C
    in_maps = []
    for c in range(N_CORES):
        sl = slice(c * B_SH, (c + 1) * B_SH)
        y_sh = y_i[sl]
        in_maps.append(
            {
                "logit": logit[sl],
                "yflat": (row_base + y_sh).reshape(B_SH, 1),
                "y32": y_sh.reshape(B_SH, 1).copy(),
                "class_occ": occ_col,
            }
        )
    return in_maps


def run_spmd(in_maps, **kwargs):
    from concourse.bass_utils import run_bass_kernel_spmd

    nc = _get_nc()
    return run_bass_kernel_spmd(nc, in_maps, core_ids=list(range(N_CORES)), **kwargs)


def kernel(logit, y, class_occ):
    res = run_spmd(make_in_maps(logit, y, class_occ))
    per_row = np.concatenate(
        [res.results[c]["loss"].reshape(B_SH) for c in range(N_CORES)]
    )
    return np.float32(np.mean(per_row))
